# revision 1
# baseline (speedup 1.0000x reference)
import sys

if "/opt/trn_rl_repo" not in sys.path:
    sys.path.insert(0, "/opt/trn_rl_repo")

import ml_dtypes
import numpy as np

import concourse.bass as bass
import concourse.bacc as bacc
import concourse.tile as tile
from concourse import bass_utils, mybir
from concourse.alu_op_type import AluOpType

C = 8          # cores
G = 1024       # segments
SPC = G // C   # segments per core
D = 256        # feature dim
H = 128        # attention hidden dim
CHUNK = 512    # nodes per pipeline chunk

F32 = mybir.dt.float32
BF16 = mybir.dt.bfloat16

_cache: dict = {}


def _build(npad: int, b2val: float):
    nchunks = npad // CHUNK
    nc = bacc.Bacc("TRN2", target_bir_lowering=False, debug=False, num_devices=C)

    x_d = nc.dram_tensor("x", [npad, D], BF16, kind="ExternalInput")
    bloc_d = nc.dram_tensor("bloc", [nchunks, 128, 4], F32, kind="ExternalInput")
    w1a_d = nc.dram_tensor("w1a", [128, H], BF16, kind="ExternalInput")
    w1b_d = nc.dram_tensor("w1b", [128, H], BF16, kind="ExternalInput")
    w2_d = nc.dram_tensor("w2", [H, 1], BF16, kind="ExternalInput")
    b1_d = nc.dram_tensor("b1", [H, 1], F32, kind="ExternalInput")
    iota_d = nc.dram_tensor("iota", [128, SPC], F32, kind="ExternalInput")
    cnt_d = nc.dram_tensor("cnt", [SPC, 1], F32, kind="ExternalInput")
    o_d = nc.dram_tensor("o", [SPC, D], F32, kind="ExternalOutput")

    TANH = mybir.ActivationFunctionType.Tanh
    EXP = mybir.ActivationFunctionType.Exp

    with tile.TileContext(nc) as tc:
        with (
            tc.tile_pool(name="const", bufs=1) as constp,
            tc.tile_pool(name="xT", bufs=3) as xTp,
            tc.tile_pool(name="th", bufs=3) as thp,
            tc.tile_pool(name="eb", bufs=3) as ebp,
            tc.tile_pool(name="bl", bufs=3) as blp,
            tc.tile_pool(name="sw", bufs=4) as swp,
            tc.tile_pool(name="xn", bufs=6) as xnp,
            tc.tile_pool(name="fin", bufs=1) as finp,
            tc.tile_pool(name="ph", bufs=2, space="PSUM") as php,
            tc.tile_pool(name="ps", bufs=2, space="PSUM") as psp,
            tc.tile_pool(name="po", bufs=1, space="PSUM") as pop,
        ):
            w1a = constp.tile([128, H], BF16)
            nc.sync.dma_start(w1a[:], w1a_d[:])
            w1b = constp.tile([128, H], BF16)
            nc.sync.dma_start(w1b[:], w1b_d[:])
            w2 = constp.tile([H, 1], BF16)
            nc.sync.dma_start(w2[:], w2_d[:])
            b1 = constp.tile([H, 1], F32)
            nc.sync.dma_start(b1[:], b1_d[:])
            iota = constp.tile([128, SPC], F32)
            nc.sync.dma_start(iota[:], iota_d[:])
            cnt = constp.tile([SPC, 1], F32)
            nc.sync.dma_start(cnt[:], cnt_d[:])

            psum_o = pop.tile([SPC, D + 1], F32)

            for t in range(nchunks):
                r0 = t * CHUNK
                # transposed read of the chunk: [128 feat, CHUNK nodes] per half
                xT = xTp.tile([128, 2, CHUNK], BF16)
                nc.sync.dma_start_transpose(xT[:, 0, :], x_d[r0 : r0 + CHUNK, 0:128])
                nc.sync.dma_start_transpose(xT[:, 1, :], x_d[r0 : r0 + CHUNK, 128:256])

                ph = php.tile([H, CHUNK], F32)
                nc.tensor.matmul(ph[:], w1a[:], xT[:, 0, :], start=True, stop=False)
                nc.tensor.matmul(ph[:], w1b[:], xT[:, 1, :], start=False, stop=True)

                th = thp.tile([H, CHUNK], BF16)
                nc.scalar.activation(th[:], ph[:], TANH, bias=b1[:], scale=1.0)

                ps = psp.tile([128, 4], F32)
                for j in range(4):
                    nc.tensor.matmul(
                        ps[:, j : j + 1],
                        th[:, j * 128 : (j + 1) * 128],
                        w2[:],
                        start=True,
                        stop=True,
                    )
                eb = ebp.tile([128, 4], F32)
                nc.scalar.activation(eb[:], ps[:], EXP, bias=b2val, scale=1.0)

                bl = blp.tile([128, 4], F32)
                nc.sync.dma_start(bl[:], bloc_d[t])

                for j in range(4):
                    sw = swp.tile([128, SPC], BF16)
                    nc.vector.tensor_scalar(
                        sw[:],
                        iota[:],
                        bl[:, j : j + 1],
                        eb[:, j : j + 1],
                        AluOpType.is_equal,
                        AluOpType.mult,
                    )
                    xn = xnp.tile([128, D + 1], BF16)
                    nc.gpsimd.memset(xn[:, D : D + 1], 1.0)
                    nc.sync.dma_start(
                        xn[:, 0:D], x_d[r0 + j * 128 : r0 + (j + 1) * 128, :]
                    )
                    nc.tensor.matmul(
                        psum_o[:],
                        sw[:],
                        xn[:],
                        start=(t == 0 and j == 0),
                        stop=(t == nchunks - 1 and j == 3),
                    )

            dent = finp.tile([SPC, 1], F32)
            nc.vector.tensor_scalar(
                dent[:],
                psum_o[:, D : D + 1],
                cnt[:],
                1e-30,
                AluOpType.mult,
                AluOpType.max,
            )
            rec = finp.tile([SPC, 1], F32)
            nc.vector.reciprocal(rec[:], dent[:])
            osb = finp.tile([SPC, D], F32)
            nc.vector.tensor_scalar_mul(osb[:], psum_o[:, 0:D], rec[:])
            nc.sync.dma_start(o_d[:], osb[:])

    nc.compile()
    return nc


def kernel(x, batch, W1, b1, W2, b2):
    x = np.asarray(x)
    batch = np.asarray(batch)
    in_dtype = batch.dtype
    W1 = np.asarray(W1, np.float32)
    b1 = np.asarray(b1, np.float32)
    W2 = np.asarray(W2, np.float32)
    b2 = np.asarray(b2, np.float32)
    n = x.shape[0]

    bat = batch.astype(np.int64)
    # per-core node ranges: core c owns segments [c*SPC, (c+1)*SPC)
    bounds = np.searchsorted(bat, np.arange(0, G + 1, SPC), side="left")
    ncounts = np.diff(bounds)
    npad = int(-(-ncounts.max() // CHUNK) * CHUNK)
    nchunks = npad // CHUNK

    counts = np.bincount(bat, minlength=G).astype(np.float32)

    key = (npad, float(b2[0]))
    if key not in _cache:
        _cache[key] = _build(npad, float(b2[0]))
    nc = _cache[key]

    x_bf = x.astype(ml_dtypes.bfloat16)
    w1a = W1[0:128, :].astype(ml_dtypes.bfloat16)
    w1b = W1[128:256, :].astype(ml_dtypes.bfloat16)
    w2 = W2.reshape(H, 1).astype(ml_dtypes.bfloat16)
    b1c = b1.reshape(H, 1).astype(np.float32)
    iota = np.broadcast_to(
        np.arange(SPC, dtype=np.float32)[None, :], (128, SPC)
    ).copy()

    in_maps = []
    for c in range(C):
        s, e = bounds[c], bounds[c + 1]
        nct = e - s
        xc = np.zeros((npad, D), ml_dtypes.bfloat16)
        xc[:nct] = x_bf[s:e]
        blc = np.full((npad,), -1.0, np.float32)
        blc[:nct] = (bat[s:e] - c * SPC).astype(np.float32)
        blc = blc.reshape(nchunks, 4, 128).transpose(0, 2, 1).copy()
        cntc = np.maximum(counts[c * SPC : (c + 1) * SPC], 1.0).reshape(SPC, 1)
        in_maps.append(
            {
                "x": xc,
                "bloc": blc,
                "w1a": w1a,
                "w1b": w1b,
                "w2": w2,
                "b1": b1c,
                "iota": iota,
                "cnt": cntc,
            }
        )

    res = bass_utils.run_bass_kernel_spmd(nc, in_maps, core_ids=list(range(C)))
    out = np.concatenate([res.results[c]["o"] for c in range(C)], axis=0)
    return out.astype(np.float32)
